# revision 12
# baseline (speedup 1.0000x reference)
"""Trainium2 Bass kernel for nn_CardEncoder: embedding gather + shared-weight
bidirectional LSTM (final states) + dense + attention over paths.

Sharding: data-parallel over the card/batch dim. Each of the 8 cores gets
B/8 = 32 cards (= 1280 paths); embedding table, LSTM, dense, and attention
params are replicated. No cross-core communication; host concatenates the
per-core [32, 256] outputs.

Self-contained: shapes/sharding hardcoded, no sibling imports.
"""

import numpy as np
import ml_dtypes

import concourse.bass as bass
import concourse.mybir as mybir
import concourse.tile as tile
from concourse import bacc
from concourse.bass_utils import run_bass_kernel_spmd

# ---- problem dims (hardcoded per spec) ----
B, P, L = 256, 40, 12
V, E = 20000, 300
H, D = 256, 256
G4 = 4 * H                   # 1024
NCORES = 8
BC = B // NCORES             # 32 cards per core
NPATH = BC * P               # 1280 paths per core
EP = 384                     # embedding width padded to 3*128
EC = EP // 128               # 3 contraction chunks over E
HC = H // 128                # 2 contraction chunks over H
MC = G4 // 128               # 8 output chunks over 4H
PTS = [512, 512, 256]        # path tiles per core
NSUB = NPATH // 128          # 10 subtiles of 128 paths

fp16 = mybir.dt.float16
f32 = mybir.dt.float32
i16 = mybir.dt.int16
AF = mybir.ActivationFunctionType
AX = mybir.AxisListType

TRACE = False                # set by test.py to collect a profile
LAST_RESULTS = None          # BassKernelResults of the last run


def _accum_matmul(nc, out_ap, pairs):
    """Emit an accumulation group of matmuls into one PSUM tile."""
    n = len(pairs)
    for j, (lhsT, rhs) in enumerate(pairs):
        nc.tensor.matmul(out_ap, lhsT, rhs, start=(j == 0), stop=(j == n - 1))


def _build(stage=5):
    """stage: 1=gathers only, 2=+recurrence, 3=+dense/scores, 4=+softmax,
    5=full (debug bisection aid; <5 writes a stand-in to `out`)."""
    nc = bacc.Bacc("TRN2", target_bir_lowering=False, debug=False,
                   num_devices=NCORES)

    embw = nc.dram_tensor("embw", [V, EP], fp16, kind="ExternalInput").ap()
    idx = nc.dram_tensor("idx", [128, L * NPATH // 16], i16,
                         kind="ExternalInput").ap()
    wk = nc.dram_tensor("wk", [EP, G4], fp16, kind="ExternalInput").ap()
    wr = nc.dram_tensor("wr", [H, G4], fp16, kind="ExternalInput").ap()
    bcols = nc.dram_tensor("bcols", [128, MC], f32, kind="ExternalInput").ap()
    wd = nc.dram_tensor("wd", [2 * H, D], fp16, kind="ExternalInput").ap()
    bdrow = nc.dram_tensor("bdrow", [1, D], fp16, kind="ExternalInput").ap()
    ones16 = nc.dram_tensor("ones16", [1, 128], fp16, kind="ExternalInput").ap()
    attb = nc.dram_tensor("attb", [128, D], fp16, kind="ExternalInput").ap()
    onessel = nc.dram_tensor("onessel", [128, NSUB * BC], fp16,
                             kind="ExternalInput").ap()
    outt = nc.dram_tensor("out", [BC, D], f32, kind="ExternalOutput").ap()

    with tile.TileContext(nc) as tc:
        with (
            tc.tile_pool(name="const", bufs=1) as constp,
            tc.tile_pool(name="gather", bufs=2) as gatherp,
            tc.tile_pool(name="gates", bufs=3) as gp,
            tc.tile_pool(name="cpool", bufs=2) as cp,
            tc.tile_pool(name="hpool", bufs=2) as hp,
            tc.tile_pool(name="state", bufs=2) as statep,
            tc.tile_pool(name="dense", bufs=NSUB) as densep,
            tc.tile_pool(name="zp", bufs=6, space="PSUM") as zp,
            tc.tile_pool(name="pdp", bufs=1, space="PSUM") as pdp,
            tc.tile_pool(name="pop", bufs=1, space="PSUM") as pop,
            tc.tile_pool(name="dram", bufs=1, space="DRAM") as dpool,
        ):
            # ---- load constants ----
            idx_sb = constp.tile([128, L * NPATH // 16], i16, tag="idx")
            nc.sync.dma_start(idx_sb[:], idx[:])
            wk_sb = []
            for e in range(EC):
                t = constp.tile([128, G4], fp16, tag=f"wk{e}")
                nc.sync.dma_start(t[:], wk[e * 128:(e + 1) * 128, :])
                wk_sb.append(t)
            wr_sb = []
            for k in range(HC):
                t = constp.tile([128, G4], fp16, tag=f"wr{k}")
                nc.sync.dma_start(t[:], wr[k * 128:(k + 1) * 128, :])
                wr_sb.append(t)
            bcols_sb = constp.tile([128, MC], f32, tag="bcols")
            nc.sync.dma_start(bcols_sb[:], bcols[:])
            wd_sb = []
            for k in range(4):
                t = constp.tile([128, D], fp16, tag=f"wd{k}")
                nc.sync.dma_start(t[:], wd[k * 128:(k + 1) * 128, :])
                wd_sb.append(t)
            bdrow_sb = constp.tile([1, D], fp16, tag="bdrow")
            nc.sync.dma_start(bdrow_sb[:], bdrow[:])
            ones_sb = constp.tile([1, 128], fp16, tag="ones16")
            nc.sync.dma_start(ones_sb[:], ones16[:])
            attb_sb = constp.tile([128, D], fp16, tag="attb")
            nc.sync.dma_start(attb_sb[:], attb[:])
            onessel_sb = constp.tile([128, NSUB * BC], fp16, tag="onessel")
            nc.sync.dma_start(onessel_sb[:], onessel[:])

            scores_sb = constp.tile([128, NSUB], f32, tag="scores")
            dense_sb = []

            # ---- per path-tile: gather + bidirectional LSTM + dense ----
            p0 = 0
            for ti, PT in enumerate(PTS):
                nt = L * PT
                # One gather per time step: HW caps a single DMA op at 128
                # s2m ring entries = 3*num_idxs/16 + 2, i.e. num_idxs <= 672
                # (1024 idxs crashes the NEFF; 512 verified good).
                col0 = (p0 * L) // 16
                xt_ts = []
                for t in range(L):
                    xth = gatherp.tile([128, EC, PT], fp16, tag=f"xt{t}")
                    nc.gpsimd.dma_gather(
                        xth[:], embw[:],
                        idx_sb[:, col0 + t * PT // 16:col0 + (t + 1) * PT // 16],
                        num_idxs=PT, num_idxs_reg=PT, elem_size=EP,
                        transpose=True,
                    )
                    xt_ts.append(xth)

                def xslice(t, e, xt_ts=xt_ts):
                    return xt_ts[t][:, e, :]

                state = {}  # (direction, hchunk) -> final hidden tile (fp16)
                for dr in range(2 if stage >= 2 else 0):  # 0 = fwd, 1 = bwd
                    h = [None, None]
                    c = [None, None]
                    for s in range(L):
                        t = s if dr == 0 else L - 1 - s
                        zt = []
                        for m in range(MC):
                            z = zp.tile([128, PT], f32, tag="z")
                            pairs = [
                                (wk_sb[e][:, m * 128:(m + 1) * 128],
                                 xslice(t, e))
                                for e in range(EC)
                            ]
                            if s > 0:
                                pairs += [
                                    (wr_sb[k][:, m * 128:(m + 1) * 128], h[k][:])
                                    for k in range(HC)
                                ]
                            _accum_matmul(nc, z[:], pairs)
                            zt.append(z)
                        for k in range(HC):
                            si = gp.tile([128, PT], f32, tag="si")
                            sf = gp.tile([128, PT], f32, tag="sf")
                            tg = gp.tile([128, PT], f32, tag="tg")
                            so = gp.tile([128, PT], f32, tag="so")
                            nc.scalar.activation(si[:], zt[0 + k][:], AF.Sigmoid,
                                                 bias=bcols_sb[:, 0 + k:1 + k])
                            nc.scalar.activation(sf[:], zt[2 + k][:], AF.Sigmoid,
                                                 bias=bcols_sb[:, 2 + k:3 + k])
                            nc.scalar.activation(tg[:], zt[4 + k][:], AF.Tanh,
                                                 bias=bcols_sb[:, 4 + k:5 + k])
                            nc.scalar.activation(so[:], zt[6 + k][:], AF.Sigmoid,
                                                 bias=bcols_sb[:, 6 + k:7 + k])
                            cn = cp.tile([128, PT], f32, tag=f"c{k}")
                            if s == 0:
                                nc.vector.tensor_mul(cn[:], si[:], tg[:])
                            else:
                                tmp = gp.tile([128, PT], f32, tag="tmp")
                                nc.vector.tensor_mul(tmp[:], si[:], tg[:])
                                nc.vector.tensor_mul(cn[:], c[k][:], sf[:])
                                nc.vector.tensor_add(cn[:], cn[:], tmp[:])
                            tch = gp.tile([128, PT], f32, tag="tc")
                            nc.scalar.activation(tch[:], cn[:], AF.Tanh)
                            if s == L - 1:
                                hn = statep.tile([128, PT], fp16,
                                                 tag=f"st{dr}{k}")
                                state[(dr, k)] = hn
                            else:
                                hn = hp.tile([128, PT], fp16, tag=f"h{k}")
                            nc.vector.tensor_mul(hn[:], so[:], tch[:])
                            h[k] = hn
                            c[k] = cn

                # dense + per-path attention scores for this tile
                for sub in range(PT // 128 if stage >= 3 else 0):
                    g = p0 // 128 + sub
                    pd = pdp.tile([128, D], f32, tag="pd")
                    pairs = [
                        (state[(dr, k)][:, sub * 128:(sub + 1) * 128],
                         wd_sb[dr * HC + k][:])
                        for dr in range(2) for k in range(HC)
                    ]
                    pairs.append((ones_sb[:], bdrow_sb[:]))
                    _accum_matmul(nc, pd[:], pairs)
                    dsb = densep.tile([128, D], fp16, tag="dense")
                    nc.scalar.activation(dsb[:], pd[:], AF.Tanh)
                    dense_sb.append(dsb)
                    tmp2 = gp.tile([128, D], f32, tag="att")
                    nc.vector.tensor_mul(tmp2[:], dsb[:], attb_sb[:])
                    nc.vector.reduce_sum(scores_sb[:, g:g + 1], tmp2[:],
                                         axis=AX.X)
                p0 += PT

            if stage < 5:
                # debug stand-in output so every stage produces `out`
                outsb = constp.tile([BC, D], f32, tag="outsb")
                if stage >= 3:
                    nc.vector.tensor_copy(outsb[:], dense_sb[0][:BC, :])
                elif stage >= 2:
                    nc.vector.tensor_copy(outsb[:], state[(0, 0)][:BC, :D])
                else:
                    nc.vector.tensor_copy(
                        outsb[:], xt_ts[0][:BC, 0, :D])
                nc.sync.dma_start(outt[:], outsb[:])

            if stage >= 5:
                # ---- softmax over each card's 40 paths ----
                stmp = dpool.tile([NSUB, 128], f32, tag="stmp")
                nc.sync.dma_start(stmp[:].rearrange("a b -> b a"),
                                  scores_sb[:])
                scard = constp.tile([BC, P], f32, tag="scard")
                nc.sync.dma_start(
                    scard[:],
                    stmp[:].rearrange("a b -> (a b)")
                        .rearrange("(c j) -> c j", j=P))
                mx = constp.tile([BC, 1], f32, tag="mx")
                nc.vector.reduce_max(mx[:], scard[:], axis=AX.X)
                negmx = constp.tile([BC, 1], f32, tag="negmx")
                nc.vector.tensor_scalar_mul(negmx[:], mx[:], -1.0)
                ex = constp.tile([BC, P], f32, tag="ex")
                nc.scalar.activation(ex[:], scard[:], AF.Exp, bias=negmx[:])
                sm = constp.tile([BC, 1], f32, tag="sm")
                nc.vector.reduce_sum(sm[:], ex[:], axis=AX.X)
                inv = constp.tile([BC, 1], f32, tag="inv")
                nc.vector.reciprocal(inv[:], sm[:])
                wtile = constp.tile([BC, P], f32, tag="wtile")
                nc.vector.tensor_scalar_mul(wtile[:], ex[:], inv[:])
                wtmp = dpool.tile([BC, P], f32, tag="wtmp")
                nc.sync.dma_start(wtmp[:], wtile[:])
                wcols = constp.tile([128, NSUB], f32, tag="wcols")
                nc.sync.dma_start(
                    wcols[:],
                    wtmp[:].rearrange("a b -> (a b)")
                        .rearrange("(g p) -> p g", p=128))

                # ---- weighted sum over paths as a selection matmul ----
                po = pop.tile([BC, D], f32, tag="po")
                for g in range(NSUB):
                    wdt = gp.tile([128, D], fp16, tag="wdt")
                    nc.vector.tensor_scalar_mul(wdt[:], dense_sb[g][:],
                                                wcols[:, g:g + 1])
                    nc.tensor.matmul(po[:], onessel_sb[:, g * BC:(g + 1) * BC],
                                     wdt[:], start=(g == 0),
                                     stop=(g == NSUB - 1))
                outsb = constp.tile([BC, D], f32, tag="outsb")
                nc.vector.tensor_copy(outsb[:], po[:])
                nc.sync.dma_start(outt[:], outsb[:])

    nc.compile()
    return nc


_NC_CACHE = {}


def _get_nc():
    if "nc" not in _NC_CACHE:
        _NC_CACHE["nc"] = _build()
    return _NC_CACHE["nc"]


def _prep_shared(emb, Wk, Wr, b, Wd, bd, att):
    embw = np.zeros((V, EP), np.float16)
    embw[:, :E] = emb.astype(np.float16)
    wk = np.zeros((EP, G4), np.float16)
    wk[:E] = Wk.astype(np.float16)
    wr = Wr.astype(np.float16)
    bcols = b.astype(np.float32).reshape(MC, 128).T.copy()
    wd = Wd.astype(np.float16)
    bdrow = bd.astype(np.float16)[None, :]
    ones = np.ones((1, 128), np.float16)
    attb = np.tile(att.astype(np.float16)[None, :], (128, 1))
    onessel = np.zeros((128, NSUB * BC), np.float16)
    for g in range(NSUB):
        for p in range(128):
            c = (g * 128 + p) // P
            onessel[p, g * BC + c] = 1.0
    return dict(embw=embw, wk=wk, wr=wr, bcols=bcols, wd=wd, bdrow=bdrow,
                ones16=ones, attb=attb, onessel=onessel)


def _prep_idx(x_core):
    """x_core: [BC, P, L] ints -> gather index tensor [128, L*NPATH/16] int16.

    Per path tile, gather i = t*PT + pp fetches token x[p0+pp, t]; index i is
    read by the DGE from [i%16, i//16] (replicated across the 8 16-partition
    stripes)."""
    xf = x_core.reshape(NPATH, L)
    cols = []
    p0 = 0
    for PT in PTS:
        ids = xf[p0:p0 + PT].T.reshape(-1)          # order: t-major, path minor
        cols.append(ids.reshape(-1, 16).T)           # [16, L*PT/16]
        p0 += PT
    block = np.concatenate(cols, axis=1).astype(np.int16)
    return np.tile(block, (8, 1))


def kernel(x, emb, Wk, Wr, b, Wd, bd, att):
    global LAST_RESULTS
    x = np.asarray(x)
    shared = _prep_shared(np.asarray(emb), np.asarray(Wk), np.asarray(Wr),
                          np.asarray(b), np.asarray(Wd), np.asarray(bd),
                          np.asarray(att))
    nc = _get_nc()
    in_maps = []
    for ci in range(NCORES):
        m = dict(shared)
        m["idx"] = _prep_idx(x[ci * BC:(ci + 1) * BC])
        in_maps.append(m)
    res = run_bass_kernel_spmd(nc, in_maps, list(range(NCORES)), trace=TRACE)
    LAST_RESULTS = res
    out = np.concatenate([res.results[ci]["out"] for ci in range(NCORES)], 0)
    return out.astype(np.float32)
